# revision 1
# baseline (speedup 1.0000x reference)
"""GCN (EAConv) 2-layer kernel for Trainium2, 8 NeuronCores.

Math: z = A @ (relu((A @ x) @ W1 + b1)) @ W2 + b2, with
A = D^-1/2 (Adj + I) D^-1/2 (PyG GCNConv norm). Uses associativity so both
irregular aggregations run at 128-wide features.

Sharding: destination nodes -> 8 cores x `wpc` windows of 128 slots,
greedy-balanced on per-window in-edge counts split by source half (so gather
indices fit int16). Weights replicated. Mid-kernel AllGather exchanges the
intermediate t = z1 @ W2 (one [slots,128] f32 shard per core).

Aggregation: batched gpsimd.dma_gather pulls source rows (edge-slot order),
DVE builds weighted selection matrices Sel[p, c*128+j] =
(dstrel[p,c]==j)*wnorm[p,c] via broadcast-AP is_equal+mult, and the PE
accumulates Sel/G matmuls into PSUM per window.

Host-side preprocessing touches ONLY edge_index (graph structure): degrees,
norm weights, node->slot permutation, edge->slot packing, int16 index tables.
All math on x_all/W1/b1/W2/b2 runs on device.
"""
import os
import sys
import math

for _p in ("/opt/trn_rl_repo", "/root/.axon_site/_ro/trn_rl_repo"):
    if os.path.isdir(_p) and _p not in sys.path:
        sys.path.insert(0, _p)

import numpy as np
import ml_dtypes

import concourse.bass as bass
import concourse.bacc as bacc
import concourse.tile as tile
from concourse import mybir
from concourse.bass_utils import run_bass_kernel_spmd

P = 128
N_CORES = 8

# knobs (test.py may override)
TRACE = False
GROUP_W = 3          # windows per gather group
GATHER_DTYPE = "bfloat16"  # aggregation-path dtype ("float32" exact mode)
LAST = {}            # stats from last run (exec_time_ns etc.)

_CACHE = {}          # structure-key -> (nc, pre)


# ---------------------------------------------------------------- preprocess
def _preprocess(edge_index, n_nodes, n_cores=N_CORES, wpc=None):
    """Graph-structure-only preprocessing. Returns dict of per-core arrays."""
    src = np.asarray(edge_index[0]).astype(np.int64)
    dst = np.asarray(edge_index[1]).astype(np.int64)
    N = n_nodes
    half_n = N // 2
    deg = np.bincount(dst, minlength=N).astype(np.float64) + 1.0
    dinv = 1.0 / np.sqrt(deg)
    loop = np.arange(N, dtype=np.int64)
    asrc = np.concatenate([src, loop])
    adst = np.concatenate([dst, loop])
    wnorm = (dinv[asrc] * dinv[adst]).astype(np.float32)
    eh = (asrc >= half_n).astype(np.int64)

    w0 = np.bincount(adst[eh == 0], minlength=N)
    w1 = np.bincount(adst[eh == 1], minlength=N)

    if wpc is None:
        wpc = int(math.ceil(N / n_cores / P))
    nwin_half = (n_cores // 2) * wpc
    assert nwin_half * P >= half_n and nwin_half * P >= (N - half_n)

    win_of = np.empty(N, np.int64)
    pos_of = np.empty(N, np.int64)
    for h in (0, 1):
        nodes = np.nonzero((np.arange(N) >= half_n) == bool(h))[0]
        order = nodes[np.argsort(-(w0[nodes] + w1[nodes]), kind="stable")]
        s0 = np.zeros(nwin_half)
        s1 = np.zeros(nwin_half)
        cnt = np.zeros(nwin_half, np.int64)
        for n in order:
            score = np.maximum(s0 + w0[n], s1 + w1[n])
            score[cnt >= P] = np.inf
            b = int(np.argmin(score))
            win_of[n] = h * nwin_half + b
            pos_of[n] = cnt[b]
            cnt[b] += 1
            s0[b] += w0[n]
            s1[b] += w1[n]

    core_of = win_of // wpc
    win_in_core = win_of % wpc
    slot_of = core_of * (wpc * P) + win_in_core * P + pos_of
    slot_half_base = (n_cores // 2) * wpc * P

    # edge -> (core, window, half), rank within bucket
    ecore = core_of[adst]
    ewin = win_in_core[adst]
    key = (ecore * wpc + ewin) * 2 + eh
    nkeys = n_cores * wpc * 2
    counts = np.bincount(key, minlength=nkeys)
    K0 = max(1, int(math.ceil(counts[0::2].max() / P)))
    K1 = max(1, int(math.ceil(counts[1::2].max() / P)))
    ordere = np.argsort(key, kind="stable")
    starts = np.r_[0, np.cumsum(counts)[:-1]]
    rank = np.empty(len(key), np.int64)
    rank[ordere] = np.arange(len(key)) - starts[key[ordere]]

    cap0, cap1 = wpc * K0 * P, wpc * K1 * P
    i_l1 = [np.zeros((n_cores, cap0), np.int64), np.zeros((n_cores, cap1), np.int64)]
    i_l2 = [np.zeros((n_cores, cap0), np.int64), np.zeros((n_cores, cap1), np.int64)]
    dsA = [np.full((n_cores, cap0), -1.0, np.float32),
           np.full((n_cores, cap1), -1.0, np.float32)]
    wnA = [np.zeros((n_cores, cap0), np.float32), np.zeros((n_cores, cap1), np.float32)]
    for h, K in ((0, K0), (1, K1)):
        m = eh == h
        c = ecore[m]
        flat = ewin[m] * K * P + rank[m]
        i_l1[h][c, flat] = asrc[m] - (half_n if h else 0)
        i_l2[h][c, flat] = slot_of[asrc[m]] - (slot_half_base if h else 0)
        dsA[h][c, flat] = pos_of[adst[m]].astype(np.float32)
        wnA[h][c, flat] = wnorm[m]

    def wrap_idx(flat):   # flat [cap] -> [128, cap//16] int16, replicated x8
        w16 = flat.reshape(-1, 16).T.astype(np.int16)
        return np.tile(w16, (8, 1)).copy()

    def dev_cols(flat, K):  # [cap] -> [128, wpc*K]; col=chunk, row=partition
        return flat.reshape(wpc * K, P).T.copy()

    pre = {
        "N": N, "half_n": half_n, "n_cores": n_cores, "wpc": wpc,
        "K0": K0, "K1": K1, "slots_per_core": wpc * P,
        "slot_half_base": slot_half_base,
        "core_of": core_of, "slot_in_core": win_in_core * P + pos_of,
        "i0_l1": [wrap_idx(i_l1[0][c]) for c in range(n_cores)],
        "i1_l1": [wrap_idx(i_l1[1][c]) for c in range(n_cores)],
        "i0_l2": [wrap_idx(i_l2[0][c]) for c in range(n_cores)],
        "i1_l2": [wrap_idx(i_l2[1][c]) for c in range(n_cores)],
        "ds0": [dev_cols(dsA[0][c], K0).astype(np.float32) for c in range(n_cores)],
        "ds1": [dev_cols(dsA[1][c], K1).astype(np.float32) for c in range(n_cores)],
        "wn0": [dev_cols(wnA[0][c], K0).astype(np.float32) for c in range(n_cores)],
        "wn1": [dev_cols(wnA[1][c], K1).astype(np.float32) for c in range(n_cores)],
    }
    return pre


# ------------------------------------------------------------------- builder
def _build(pre, D, H, gdt_name=None):
    N = pre["N"]
    half_n = pre["half_n"]
    n_cores = pre["n_cores"]
    wpc = pre["wpc"]
    K0, K1 = pre["K0"], pre["K1"]
    C = K0 + K1
    spc = pre["slots_per_core"]
    shb = pre["slot_half_base"]
    JH = H // P  # 4
    f32 = mybir.dt.float32
    gdt = getattr(mybir.dt, gdt_name or GATHER_DTYPE)
    gsz = mybir.dt.size(gdt)

    nc = bacc.Bacc("TRN2", target_bir_lowering=False, debug=False,
                   num_devices=n_cores, num_swdge_queues=4)
    qctr = [0]

    def next_q():
        q = qctr[0] % 4
        qctr[0] += 1
        return q
    x = nc.dram_tensor("x_all", [N, D], f32, kind="ExternalInput").ap()
    W1 = nc.dram_tensor("W1", [D, H], f32, kind="ExternalInput").ap()
    b1c = nc.dram_tensor("b1c", [P, JH], f32, kind="ExternalInput").ap()
    W2 = nc.dram_tensor("W2", [H, D], f32, kind="ExternalInput").ap()
    b2r = nc.dram_tensor("b2r", [1, D], f32, kind="ExternalInput").ap()
    iota_in = nc.dram_tensor("iota_in", [P, P], gdt, kind="ExternalInput").ap()
    i0l1 = nc.dram_tensor("i0l1", [P, wpc * K0 * 8], mybir.dt.int16, kind="ExternalInput").ap()
    i1l1 = nc.dram_tensor("i1l1", [P, wpc * K1 * 8], mybir.dt.int16, kind="ExternalInput").ap()
    i0l2 = nc.dram_tensor("i0l2", [P, wpc * K0 * 8], mybir.dt.int16, kind="ExternalInput").ap()
    i1l2 = nc.dram_tensor("i1l2", [P, wpc * K1 * 8], mybir.dt.int16, kind="ExternalInput").ap()
    ds0 = nc.dram_tensor("ds0", [P, wpc * K0], gdt, kind="ExternalInput").ap()
    ds1 = nc.dram_tensor("ds1", [P, wpc * K1], gdt, kind="ExternalInput").ap()
    wn0 = nc.dram_tensor("wn0", [P, wpc * K0], gdt, kind="ExternalInput").ap()
    wn1 = nc.dram_tensor("wn1", [P, wpc * K1], gdt, kind="ExternalInput").ap()
    z_out = nc.dram_tensor("z_out", [spc, D], f32, kind="ExternalOutput").ap()

    groups = []
    a = 0
    while a < wpc:
        b = min(a + GROUP_W, wpc)
        groups.append((a, b))
        a = b

    with tile.TileContext(nc) as tc:
        with tc.tile_pool(name="const", bufs=1) as cst, \
             tc.tile_pool(name="big", bufs=3) as big, \
             tc.tile_pool(name="sm", bufs=3) as sm, \
             tc.tile_pool(name="selp", bufs=8) as selp, \
             tc.tile_pool(name="work", bufs=2) as wk, \
             tc.tile_pool(name="ps", bufs=2, space="PSUM") as ps, \
             tc.tile_pool(name="dram", bufs=1, space="DRAM") as dram:

            # constants
            W1_sb = cst.tile([P, H], f32)
            nc.sync.dma_start(out=W1_sb[:], in_=W1)
            W2_sb = cst.tile([P, JH * D], f32)
            for j in range(JH):
                nc.sync.dma_start(out=W2_sb[:, j * D:(j + 1) * D],
                                  in_=W2[j * P:(j + 1) * P, :])
            b1_sb = cst.tile([P, JH], f32)
            nc.sync.dma_start(out=b1_sb[:], in_=b1c)
            b2_sb = cst.tile([1, D], f32)
            nc.sync.dma_start(out=b2_sb[:], in_=b2r)
            iota_sb = cst.tile([P, P], gdt)
            nc.sync.dma_start(out=iota_sb[:], in_=iota_in)
            ones_sb = cst.tile([1, P], gdt)
            nc.vector.memset(ones_sb[:], 1.0)
            if gdt != f32:
                b2g_sb = cst.tile([1, D], gdt)
                nc.vector.tensor_copy(out=b2g_sb[:], in_=b2_sb[:])
            else:
                b2g_sb = b2_sb

            cc_in = dram.tile([spc, D], gdt)
            cc_out = dram.tile([n_cores * spc, D], gdt, addr_space="Shared")

            def agg_phase(layer, idx0_d, idx1_d, tbl0, tbl1, emit_window):
                """One aggregation sweep over all windows.
                emit_window(w, psum_u) consumes the window's aggregate."""
                for (a, b) in groups:
                    W_g = b - a
                    n0, n1 = W_g * K0 * P, W_g * K1 * P
                    i0_sb = sm.tile([P, GROUP_W * K0 * 8], mybir.dt.int16, tag="i0")
                    nc.sync.dma_start(out=i0_sb[:, :W_g * K0 * 8],
                                      in_=idx0_d[:, a * K0 * 8:b * K0 * 8])
                    i1_sb = sm.tile([P, GROUP_W * K1 * 8], mybir.dt.int16, tag="i1")
                    nc.sync.dma_start(out=i1_sb[:, :W_g * K1 * 8],
                                      in_=idx1_d[:, a * K1 * 8:b * K1 * 8])
                    ds0_sb = sm.tile([P, GROUP_W * K0], gdt, tag="ds0")
                    nc.sync.dma_start(out=ds0_sb[:, :W_g * K0],
                                      in_=ds0[:, a * K0:b * K0])
                    ds1_sb = sm.tile([P, GROUP_W * K1], gdt, tag="ds1")
                    nc.sync.dma_start(out=ds1_sb[:, :W_g * K1],
                                      in_=ds1[:, a * K1:b * K1])
                    wn0_sb = sm.tile([P, GROUP_W * K0], gdt, tag="wn0")
                    nc.sync.dma_start(out=wn0_sb[:, :W_g * K0],
                                      in_=wn0[:, a * K0:b * K0])
                    wn1_sb = sm.tile([P, GROUP_W * K1], gdt, tag="wn1")
                    nc.sync.dma_start(out=wn1_sb[:, :W_g * K1],
                                      in_=wn1[:, a * K1:b * K1])

                    G0 = big.tile([P, GROUP_W * K0 * P], gdt, tag="G0")
                    G1 = big.tile([P, GROUP_W * K1 * P], gdt, tag="G1")
                    for (Gt, tbl, isb, nch) in ((G0, tbl0, i0_sb, W_g * K0),
                                                (G1, tbl1, i1_sb, W_g * K1)):
                        ca = max(1, nch // 2)
                        for (c_lo, c_hi) in ((0, ca), (ca, nch)):
                            if c_hi <= c_lo:
                                continue
                            nn = (c_hi - c_lo) * P
                            nc.gpsimd.dma_gather(
                                out_ap=Gt[:, c_lo * P * P // P:c_hi * P * P // P]
                                    .rearrange("p (k d) -> p k d", d=P),
                                in_ap=tbl,
                                idxs_ap=isb[:, c_lo * 8:c_hi * 8],
                                num_idxs=nn, num_idxs_reg=nn, elem_size=P,
                                single_packet=False, queue_num=next_q())

                    sel0 = big.tile([P, GROUP_W * K0 * P], gdt, tag="sel0")
                    sel1 = big.tile([P, GROUP_W * K1 * P], gdt, tag="sel1")
                    for (nn, dss, wns, st) in (
                            (W_g * K0, ds0_sb, wn0_sb, sel0),
                            (W_g * K1, ds1_sb, wn1_sb, sel1)):
                        s3 = st[:, :nn * P].rearrange("p (c j) -> p c j", j=P)
                        d_b = dss[:, :nn].unsqueeze(2).broadcast_to([P, nn, P])
                        i_b = iota_sb[:].unsqueeze(1).broadcast_to([P, nn, P])
                        w_b = wns[:, :nn].unsqueeze(2).broadcast_to([P, nn, P])
                        nc.vector.tensor_tensor(out=s3, in0=d_b, in1=i_b,
                                                op=mybir.AluOpType.is_equal)
                        nc.vector.tensor_tensor(out=s3, in0=s3, in1=w_b,
                                                op=mybir.AluOpType.mult)

                    for wl in range(W_g):
                        w = a + wl
                        psum_u = ps.tile([P, P], f32, tag="psum_u")
                        chunks = [(G0, sel0, (wl * K0 + k) * P) for k in range(K0)] + \
                                 [(G1, sel1, (wl * K1 + k) * P) for k in range(K1)]
                        for ci, (Gt, st, off) in enumerate(chunks):
                            gsl = Gt[:, off:off + P]
                            ssl = st[:, off:off + P]
                            if layer == 1:
                                nc.tensor.matmul(psum_u[:], lhsT=gsl, rhs=ssl,
                                                 start=(ci == 0),
                                                 stop=(ci == C - 1))
                            else:
                                nc.tensor.matmul(psum_u[:], lhsT=ssl, rhs=gsl,
                                                 start=(ci == 0), stop=False)
                        emit_window(w, psum_u)

            # ---- phase 1: u' = (A x)^T per window -> z1 -> t -> cc_in
            def emit_l1(w, psum_u):
                u_sb = wk.tile([P, P], f32, tag="u")
                nc.vector.tensor_copy(out=u_sb[:], in_=psum_u[:])
                psum_z1 = ps.tile([P, H], f32, tag="psum_z1")
                z1_sb = wk.tile([P, H], f32, tag="z1")
                for j in range(JH):
                    nc.tensor.matmul(psum_z1[:, j * P:(j + 1) * P],
                                     lhsT=W1_sb[:, j * P:(j + 1) * P],
                                     rhs=u_sb[:], start=True, stop=True)
                    nc.scalar.activation(out=z1_sb[:, j * P:(j + 1) * P],
                                         in_=psum_z1[:, j * P:(j + 1) * P],
                                         func=mybir.ActivationFunctionType.Relu,
                                         bias=b1_sb[:, j:j + 1])
                psum_t = ps.tile([P, P], f32, tag="psum_t")
                for j in range(JH):
                    nc.tensor.matmul(psum_t[:],
                                     lhsT=z1_sb[:, j * P:(j + 1) * P],
                                     rhs=W2_sb[:, j * D:(j + 1) * D],
                                     start=(j == 0), stop=(j == JH - 1))
                t_sb = wk.tile([P, D], gdt, tag="t")
                nc.vector.tensor_copy(out=t_sb[:], in_=psum_t[:])
                nc.sync.dma_start(out=cc_in[w * P:(w + 1) * P, :], in_=t_sb[:])

            if gdt == f32:
                tbl0_l1, tbl1_l1 = x[:half_n, :], x[half_n:, :]
            else:
                xg = dram.tile([N, D], gdt)
                nrb = (N + 3) // 4
                r0 = 0
                while r0 < N:
                    r1 = min(r0 + nrb, N)
                    nc.gpsimd.dma_start(out=xg[r0:r1, :], in_=x[r0:r1, :])
                    r0 = r1
                tbl0_l1, tbl1_l1 = xg[:half_n, :], xg[half_n:, :]

            agg_phase(1, i0l1, i1l1, tbl0_l1, tbl1_l1, emit_l1)

            nc.gpsimd.collective_compute(
                "AllGather", mybir.AluOpType.bypass,
                replica_groups=[list(range(n_cores))],
                ins=[cc_in[:]], outs=[cc_out[:]])

            cc_g = cc_out

            # ---- phase 2: z = (A t) + b2 per window -> z_out
            def emit_l2(w, psum_u):
                nc.tensor.matmul(psum_u[:], lhsT=ones_sb[:], rhs=b2g_sb[:],
                                 start=False, stop=True)
                zw_sb = wk.tile([P, D], f32, tag="zw")
                nc.vector.tensor_copy(out=zw_sb[:], in_=psum_u[:])
                nc.sync.dma_start(out=z_out[w * P:(w + 1) * P, :], in_=zw_sb[:])

            agg_phase(2, i0l2, i1l2, cc_g[:shb, :], cc_g[shb:, :], emit_l2)

    nc.compile()
    return nc


# -------------------------------------------------------------------- kernel
def kernel(x_all, W1, b1, W2, b2, edge_index, ix=0, max_iter=10):
    x_all = np.ascontiguousarray(np.asarray(x_all, dtype=np.float32))
    W1 = np.ascontiguousarray(np.asarray(W1, dtype=np.float32))
    b1 = np.ascontiguousarray(np.asarray(b1, dtype=np.float32))
    W2 = np.ascontiguousarray(np.asarray(W2, dtype=np.float32))
    b2 = np.ascontiguousarray(np.asarray(b2, dtype=np.float32))
    edge_index = np.asarray(edge_index)

    N, D = x_all.shape
    H = W1.shape[1]
    ekey = (N, D, H, edge_index.shape[1], GATHER_DTYPE, GROUP_W,
            int(edge_index[0, 0]), int(edge_index[1, -1]))
    if ekey in _CACHE:
        nc, pre = _CACHE[ekey]
    else:
        pre = _preprocess(edge_index, N)
        nc = _build(pre, D, H)
        _CACHE[ekey] = (nc, pre)

    JH = H // P
    b1c = b1.reshape(JH, P).T.copy()          # [128, JH]
    b2r = b2.reshape(1, D).copy()
    gnp = np.float32 if GATHER_DTYPE == "float32" else ml_dtypes.bfloat16
    iota = np.broadcast_to(np.arange(P, dtype=np.float32), (P, P)).astype(gnp)

    in_maps = []
    for c in range(pre["n_cores"]):
        in_maps.append({
            "x_all": x_all, "W1": W1, "b1c": b1c, "W2": W2, "b2r": b2r,
            "iota_in": iota,
            "i0l1": pre["i0_l1"][c], "i1l1": pre["i1_l1"][c],
            "i0l2": pre["i0_l2"][c], "i1l2": pre["i1_l2"][c],
            "ds0": pre["ds0"][c].astype(gnp), "ds1": pre["ds1"][c].astype(gnp),
            "wn0": pre["wn0"][c].astype(gnp), "wn1": pre["wn1"][c].astype(gnp),
        })

    res = run_bass_kernel_spmd(nc, in_maps, core_ids=list(range(pre["n_cores"])),
                               trace=TRACE)
    LAST["exec_time_ns"] = res.exec_time_ns
    LAST["mean_exec_time_ns"] = res.mean_exec_time_ns
    LAST["per_core_scope_times"] = res.per_core_scope_times
    LAST["trace_path"] = (res.instructions_and_trace or (None, None))[1]
    LAST["profile_json"] = res.profile_json

    zs = np.stack([res.results[c]["z_out"] for c in range(pre["n_cores"])])
    z = zs[pre["core_of"], pre["slot_in_core"]]
    return z.astype(np.float32)


if __name__ == "__main__":
    # small smoke test against numpy reference
    rng = np.random.default_rng(0)
    N, E, D, H = 4096, 40000, 128, 512
    ei = rng.integers(0, N, size=(2, E)).astype(np.int64)
    x = rng.standard_normal((N, D), dtype=np.float32)
    W1 = rng.standard_normal((D, H), dtype=np.float32) / np.sqrt(D)
    b1 = rng.standard_normal(H).astype(np.float32) * 0.1
    W2 = rng.standard_normal((H, D), dtype=np.float32) / np.sqrt(H)
    b2 = rng.standard_normal(D).astype(np.float32) * 0.1

    deg = np.bincount(ei[1], minlength=N) + 1.0
    dinv = 1.0 / np.sqrt(deg)
    asrc = np.concatenate([ei[0], np.arange(N)])
    adst = np.concatenate([ei[1], np.arange(N)])
    nrm = dinv[asrc] * dinv[adst]

    def agg(t):
        out = np.zeros_like(t)
        np.add.at(out, adst, t[asrc] * nrm[:, None])
        return out

    z1 = np.maximum(agg(x.astype(np.float64)) @ W1 + b1, 0)
    ref = agg(z1 @ W2) + b2

    got = kernel(x, W1, b1, W2, b2, ei)
    err = np.abs(got - ref)
    rel = err.max() / np.abs(ref).max()
    print(f"exec_time_ns: {LAST['exec_time_ns']}")
    print(f"max abs err {err.max():.3e}  rel(absmax) {rel:.3e}")



# revision 5
# speedup vs baseline: 1.4139x; 1.4139x over previous
"""GCN (EAConv) 2-layer kernel for Trainium2, 8 NeuronCores — v2.

Math: z = A @ relu((A @ x) @ W1 + b1) @ W2 + b2, A = D^-1/2 (Adj+I) D^-1/2.
Associativity keeps both aggregations at 128-wide features.

v2 design (vs v1):
- Unified slot space: x rows are host-permuted into destination-slot order, so
  phase 1 (aggregate x) and phase 2 (aggregate t) share ONE set of int16 gather
  tables and ONE set of selection masks.
- Weight factorization w_e = dinv[src]*dinv[dst]: dinv[src] is folded into the
  gather table at cast time (on device), dinv[dst] is carried by the host-built
  {0, dinv}-valued selection masks streamed from HBM (no DVE is_equal builds).
- Self-loop edges use the window's own contiguous rows (kept in SBUF from the
  cast / t computation) — no random gather for them.
- dma_gather calls round-robin the 4 SWDGE queues (descgen runs on the Q7 core
  pair selected by queue_num, so 4 queues give ~4x descriptor throughput).
- Sharded bf16 cast + AllGather replaces the per-core full-x cast; a second
  AllGather publishes t between phases.

Host-side preprocessing touches ONLY edge_index (graph structure): degrees,
node->slot permutation, edge->chunk packing, int16 index tables, masks.
All math on x_all/W1/b1/W2/b2 runs on device (incl. the dinv prescale + casts).
"""
import os
import sys
import math

for _p in ("/opt/trn_rl_repo", "/root/.axon_site/_ro/trn_rl_repo"):
    if os.path.isdir(_p) and _p not in sys.path:
        sys.path.insert(0, _p)

import numpy as np
import ml_dtypes

import concourse.bass as bass
import concourse.bacc as bacc
import concourse.tile as tile
from concourse import mybir
from concourse.bass_utils import run_bass_kernel_spmd

P = 128
N_CORES = 8

# knobs (test.py may override)
TRACE = False
GROUP_W = 4          # windows per gather/mask group
LAST = {}            # stats from last run (exec_time_ns etc.)

_CACHE = {}          # structure-key -> (nc, pre)


# ---------------------------------------------------------------- preprocess
def _preprocess(edge_index, n_nodes, n_cores=N_CORES, wpc=None):
    """Graph-structure-only preprocessing. Returns dict of per-core arrays."""
    src = np.asarray(edge_index[0]).astype(np.int64)
    dst = np.asarray(edge_index[1]).astype(np.int64)
    N = n_nodes
    half_n = N // 2
    deg = np.bincount(dst, minlength=N).astype(np.float64) + 1.0
    dinv = (1.0 / np.sqrt(deg)).astype(np.float32)
    eh = (src >= half_n).astype(np.int64)   # src half (self loops excluded)

    w0 = np.bincount(dst[eh == 0], minlength=N)
    w1 = np.bincount(dst[eh == 1], minlength=N)

    if wpc is None:
        wpc = int(math.ceil(N / n_cores / P))
    nwin_half = (n_cores // 2) * wpc
    assert nwin_half * P >= half_n and nwin_half * P >= (N - half_n)

    # greedy balance of nodes into windows (per half), minimizing the max
    # per-(window, src-half) in-edge count
    win_of = np.empty(N, np.int64)
    pos_of = np.empty(N, np.int64)
    for h in (0, 1):
        nodes = np.nonzero((np.arange(N) >= half_n) == bool(h))[0]
        order = nodes[np.argsort(-(w0[nodes] + w1[nodes]), kind="stable")]
        s0 = np.zeros(nwin_half)
        s1 = np.zeros(nwin_half)
        cnt = np.zeros(nwin_half, np.int64)
        for n in order:
            score = np.maximum(s0 + w0[n], s1 + w1[n])
            score[cnt >= P] = np.inf
            b = int(np.argmin(score))
            win_of[n] = h * nwin_half + b
            pos_of[n] = cnt[b]
            cnt[b] += 1
            s0[b] += w0[n]
            s1[b] += w1[n]

    core_of = win_of // wpc
    win_in_core = win_of % wpc
    spc = wpc * P
    slot_of = core_of * spc + win_in_core * P + pos_of
    SHB = (n_cores // 2) * spc

    # edge -> (window, src-half) bucket; rank within bucket
    key = win_of[dst] * 2 + eh
    nkeys = n_cores * wpc * 2
    counts = np.bincount(key, minlength=nkeys)
    K0 = max(1, int(math.ceil(counts[0::2].max() / P)))
    K1 = max(1, int(math.ceil(counts[1::2].max() / P)))
    KT = 1 + K0 + K1
    ordere = np.argsort(key, kind="stable")
    starts = np.r_[0, np.cumsum(counts)[:-1]]
    rank = np.empty(len(key), np.int64)
    rank[ordere] = np.arange(len(key)) - starts[key[ordere]]
    c_chunk = rank // P
    p_part = rank % P

    cap0, cap1 = wpc * K0 * P, wpc * K1 * P
    i_tab = [np.zeros((n_cores, cap0), np.int64), np.zeros((n_cores, cap1), np.int64)]
    maskA = np.zeros((n_cores, P, wpc * KT * P), np.float32)

    ecore = core_of[dst]
    ewin = win_in_core[dst]
    for h, K in ((0, K0), (1, K1)):
        m = eh == h
        relidx = slot_of[src[m]] - SHB * h
        assert relidx.min() >= 0 and relidx.max() < 32768
        flat = (ewin[m] * K + c_chunk[m]) * P + p_part[m]
        i_tab[h][ecore[m], flat] = relidx
        col = (ewin[m] * KT + 1 + (K0 if h == 1 else 0) + c_chunk[m]) * P + pos_of[dst[m]]
        maskA[ecore[m], p_part[m], col] = dinv[dst[m]]

    # self-loop diagonal chunk (chunk 0 of each window)
    colself = (win_in_core * KT) * P + pos_of
    maskA[core_of, pos_of, colself] = dinv

    # slot -> node map; dcol (dinv laid out [p, w] per core); x permutation
    tot = n_cores * spc
    slot2node = np.full(tot, -1, np.int64)
    slot2node[slot_of] = np.arange(N)
    s = np.arange(tot)
    core_s = s // spc
    rem = s % spc
    w_s = rem // P
    p_s = rem % P
    valid = slot2node >= 0
    dcol = np.zeros((n_cores, P, wpc), np.float32)
    dcol[core_s[valid], p_s[valid], w_s[valid]] = dinv[slot2node[valid]]
    xidx = np.where(valid, slot2node, 0).reshape(n_cores, spc)

    def wrap_idx(flat):   # flat [cap] -> [128, cap//16] int16, replicated x8
        w16 = flat.reshape(-1, 16).T.astype(np.int16)
        return np.tile(w16, (8, 1)).copy()

    pre = {
        "N": N, "n_cores": n_cores, "wpc": wpc, "spc": spc, "SHB": SHB,
        "K0": K0, "K1": K1, "KT": KT,
        "core_of": core_of, "slot_in_core": win_in_core * P + pos_of,
        "xidx": xidx, "xvalid": valid.reshape(n_cores, spc),
        "i0": [wrap_idx(i_tab[0][c]) for c in range(n_cores)],
        "i1": [wrap_idx(i_tab[1][c]) for c in range(n_cores)],
        "mask": [maskA[c].astype(ml_dtypes.bfloat16) for c in range(n_cores)],
        "dcol": [dcol[c] for c in range(n_cores)],
    }
    return pre


# ------------------------------------------------------------------- builder
def _build(pre, D, H):
    n_cores = pre["n_cores"]
    wpc = pre["wpc"]
    spc = pre["spc"]
    SHB = pre["SHB"]
    K0, K1, KT = pre["K0"], pre["K1"], pre["KT"]
    JH = H // P  # 4
    f32 = mybir.dt.float32
    bf16 = mybir.dt.bfloat16

    nc = bacc.Bacc("TRN2", target_bir_lowering=False, debug=False,
                   num_devices=n_cores, num_swdge_queues=4)

    xs = nc.dram_tensor("xs", [spc, D], f32, kind="ExternalInput").ap()
    W1 = nc.dram_tensor("W1", [D, H], f32, kind="ExternalInput").ap()
    b1c = nc.dram_tensor("b1c", [P, JH], f32, kind="ExternalInput").ap()
    W2 = nc.dram_tensor("W2", [H, D], f32, kind="ExternalInput").ap()
    b2r = nc.dram_tensor("b2r", [1, D], f32, kind="ExternalInput").ap()
    i0_d = nc.dram_tensor("i0", [P, wpc * K0 * 8], mybir.dt.int16, kind="ExternalInput").ap()
    i1_d = nc.dram_tensor("i1", [P, wpc * K1 * 8], mybir.dt.int16, kind="ExternalInput").ap()
    mask_d = nc.dram_tensor("mask", [P, wpc * KT * P], bf16, kind="ExternalInput").ap()
    dcol_d = nc.dram_tensor("dcol", [P, wpc], f32, kind="ExternalInput").ap()
    z_out = nc.dram_tensor("z_out", [spc, D], f32, kind="ExternalOutput").ap()

    groups = []
    a = 0
    while a < wpc:
        b = min(a + GROUP_W, wpc)
        groups.append((a, b))
        a = b

    with tile.TileContext(nc) as tc:
        with tc.tile_pool(name="cst", bufs=1) as cst, \
             tc.tile_pool(name="mp", bufs=2) as mp, \
             tc.tile_pool(name="gp", bufs=2) as gp, \
             tc.tile_pool(name="wk", bufs=3) as wk, \
             tc.tile_pool(name="ps", bufs=2, space="PSUM") as ps, \
             tc.tile_pool(name="dram", bufs=1, space="DRAM") as dram:

            # ---- constants (load + on-device bf16 casts of weights)
            W1f = cst.tile([P, H], f32)
            nc.sync.dma_start(out=W1f[:], in_=W1)
            W1_sb = cst.tile([P, H], bf16)
            nc.vector.tensor_copy(out=W1_sb[:], in_=W1f[:])
            W2f = cst.tile([P, JH * D], f32)
            for j in range(JH):
                nc.sync.dma_start(out=W2f[:, j * D:(j + 1) * D],
                                  in_=W2[j * P:(j + 1) * P, :])
            W2_sb = cst.tile([P, JH * D], bf16)
            nc.vector.tensor_copy(out=W2_sb[:], in_=W2f[:])
            b1_sb = cst.tile([P, JH], f32)
            nc.sync.dma_start(out=b1_sb[:], in_=b1c)
            b2f = cst.tile([1, D], f32)
            nc.sync.dma_start(out=b2f[:], in_=b2r)
            b2_sb = cst.tile([1, D], bf16)
            nc.vector.tensor_copy(out=b2_sb[:], in_=b2f[:])
            ones_sb = cst.tile([1, P], bf16)
            nc.vector.memset(ones_sb[:], 1.0)
            dcol_sb = cst.tile([P, wpc], f32)
            nc.sync.dma_start(out=dcol_sb[:], in_=dcol_d)
            i0_sb = cst.tile([P, wpc * K0 * 8], mybir.dt.int16)
            nc.sync.dma_start(out=i0_sb[:], in_=i0_d)
            i1_sb = cst.tile([P, wpc * K1 * 8], mybir.dt.int16)
            nc.sync.dma_start(out=i1_sb[:], in_=i1_d)

            xc_sb = cst.tile([P, spc], bf16)   # dinv-scaled x, window-major
            tc_sb = cst.tile([P, spc], bf16)   # t, window-major
            xin_sb = cst.tile([P, spc], f32)

            cc_x = dram.tile([spc, D], bf16)
            xg = dram.tile([n_cores * spc, D], bf16, addr_space="Shared")
            cc_t = dram.tile([spc, D], bf16)
            tg = dram.tile([n_cores * spc, D], bf16, addr_space="Shared")

            # ---- phase 0: prescale by dinv, cast to bf16, AllGather
            nc.sync.dma_start(out=xin_sb[:].rearrange("p (w f) -> p w f", f=D),
                              in_=xs.rearrange("(w p) f -> p w f", p=P))
            for w in range(wpc):
                nc.scalar.mul(out=xc_sb[:, w * P:(w + 1) * P],
                              in_=xin_sb[:, w * P:(w + 1) * P],
                              mul=dcol_sb[:, w:w + 1])
            nc.sync.dma_start(out=cc_x[:].rearrange("(w p) f -> p w f", p=P),
                              in_=xc_sb[:].rearrange("p (w f) -> p w f", f=D))
            nc.gpsimd.collective_compute(
                "AllGather", mybir.AluOpType.bypass,
                replica_groups=[list(range(n_cores))],
                ins=[cc_x[:]], outs=[xg[:]])

            # ---- aggregation phases
            def agg_phase(phase):
                tbl = xg if phase == 1 else tg
                selfsb = xc_sb if phase == 1 else tc_sb
                for gi, (a, b) in enumerate(groups):
                    W_g = b - a
                    mk = mp.tile([P, GROUP_W * KT * P], bf16, tag="mk")
                    nc.sync.dma_start(out=mk[:, :W_g * KT * P],
                                      in_=mask_d[:, a * KT * P:b * KT * P])
                    G0t = gp.tile([P, GROUP_W * K0 * P], bf16, tag="G0")
                    G1t = gp.tile([P, GROUP_W * K1 * P], bf16, tag="G1")
                    for (Gt, K, isb, tsl, qoff) in (
                            (G0t, K0, i0_sb, tbl[:SHB, :], 0),
                            (G1t, K1, i1_sb, tbl[SHB:, :], 2)):
                        nch = W_g * K
                        ca = (nch + 1) // 2
                        for si, (lo, hi) in enumerate(((0, ca), (ca, nch))):
                            if hi <= lo:
                                continue
                            nn = (hi - lo) * P
                            nc.gpsimd.dma_gather(
                                out_ap=Gt[:, lo * P:hi * P]
                                    .rearrange("p (k d) -> p k d", d=P),
                                in_ap=tsl,
                                idxs_ap=isb[:, (a * K + lo) * 8:(a * K + hi) * 8],
                                num_idxs=nn, num_idxs_reg=nn, elem_size=P,
                                single_packet=False, queue_num=qoff + si)

                    for wl in range(W_g):
                        w = a + wl
                        psum_u = ps.tile([P, P], f32, tag="pu")
                        mwin = mk[:, wl * KT * P:(wl + 1) * KT * P]
                        chunks = [(selfsb[:, w * P:(w + 1) * P], mwin[:, 0:P])]
                        chunks += [(G0t[:, (wl * K0 + k) * P:(wl * K0 + k + 1) * P],
                                    mwin[:, (1 + k) * P:(2 + k) * P])
                                   for k in range(K0)]
                        chunks += [(G1t[:, (wl * K1 + k) * P:(wl * K1 + k + 1) * P],
                                    mwin[:, (1 + K0 + k) * P:(2 + K0 + k) * P])
                                   for k in range(K1)]
                        for ci, (gsl, msl) in enumerate(chunks):
                            if phase == 1:
                                nc.tensor.matmul(psum_u[:], lhsT=gsl, rhs=msl,
                                                 start=(ci == 0),
                                                 stop=(ci == KT - 1))
                            else:
                                nc.tensor.matmul(psum_u[:], lhsT=msl, rhs=gsl,
                                                 start=(ci == 0), stop=False)
                        if phase == 1:
                            u_sb = wk.tile([P, P], bf16, tag="u")
                            nc.scalar.copy(out=u_sb[:], in_=psum_u[:])
                            psum_z1 = ps.tile([P, H], f32, tag="pz")
                            z1_sb = wk.tile([P, H], bf16, tag="z1")
                            for j in range(JH):
                                nc.tensor.matmul(psum_z1[:, j * P:(j + 1) * P],
                                                 lhsT=W1_sb[:, j * P:(j + 1) * P],
                                                 rhs=u_sb[:], start=True, stop=True)
                                nc.scalar.activation(
                                    out=z1_sb[:, j * P:(j + 1) * P],
                                    in_=psum_z1[:, j * P:(j + 1) * P],
                                    func=mybir.ActivationFunctionType.Relu,
                                    bias=b1_sb[:, j:j + 1])
                            psum_t = ps.tile([P, P], f32, tag="pt")
                            for j in range(JH):
                                nc.tensor.matmul(psum_t[:],
                                                 lhsT=z1_sb[:, j * P:(j + 1) * P],
                                                 rhs=W2_sb[:, j * D:(j + 1) * D],
                                                 start=(j == 0), stop=(j == JH - 1))
                            nc.scalar.mul(out=tc_sb[:, w * P:(w + 1) * P],
                                          in_=psum_t[:],
                                          mul=dcol_sb[:, w:w + 1])
                            nc.sync.dma_start(out=cc_t[w * P:(w + 1) * P, :],
                                              in_=tc_sb[:, w * P:(w + 1) * P])
                        else:
                            nc.tensor.matmul(psum_u[:], lhsT=ones_sb[:],
                                             rhs=b2_sb[:], start=False, stop=True)
                            zw = wk.tile([P, D], f32, tag="zw")
                            nc.scalar.copy(out=zw[:], in_=psum_u[:])
                            nc.sync.dma_start(out=z_out[w * P:(w + 1) * P, :],
                                              in_=zw[:])

            agg_phase(1)

            nc.gpsimd.collective_compute(
                "AllGather", mybir.AluOpType.bypass,
                replica_groups=[list(range(n_cores))],
                ins=[cc_t[:]], outs=[tg[:]])

            agg_phase(2)

    nc.compile()
    return nc


# -------------------------------------------------------------------- kernel
def kernel(x_all, W1, b1, W2, b2, edge_index, ix=0, max_iter=10):
    x_all = np.ascontiguousarray(np.asarray(x_all, dtype=np.float32))
    W1 = np.ascontiguousarray(np.asarray(W1, dtype=np.float32))
    b1 = np.ascontiguousarray(np.asarray(b1, dtype=np.float32))
    W2 = np.ascontiguousarray(np.asarray(W2, dtype=np.float32))
    b2 = np.ascontiguousarray(np.asarray(b2, dtype=np.float32))
    edge_index = np.asarray(edge_index)

    N, D = x_all.shape
    H = W1.shape[1]
    ekey = (N, D, H, edge_index.shape[1], GROUP_W,
            int(edge_index[0, 0]), int(edge_index[1, -1]))
    if ekey in _CACHE:
        nc, pre = _CACHE[ekey]
    else:
        pre = _preprocess(edge_index, N)
        nc = _build(pre, D, H)
        _CACHE[ekey] = (nc, pre)

    JH = H // P
    b1cm = b1.reshape(JH, P).T.copy()          # [128, JH]
    b2rm = b2.reshape(1, D).copy()

    # permuted, zero-padded x slices (pure data movement / sharding)
    in_maps = []
    for c in range(pre["n_cores"]):
        xsl = x_all[pre["xidx"][c]]
        xsl = xsl * pre["xvalid"][c][:, None]  # zero the pad slots
        in_maps.append({
            "xs": np.ascontiguousarray(xsl, dtype=np.float32),
            "W1": W1, "b1c": b1cm, "W2": W2, "b2r": b2rm,
            "i0": pre["i0"][c], "i1": pre["i1"][c],
            "mask": pre["mask"][c], "dcol": pre["dcol"][c],
        })

    res = run_bass_kernel_spmd(nc, in_maps, core_ids=list(range(pre["n_cores"])),
                               trace=TRACE)
    LAST["exec_time_ns"] = res.exec_time_ns
    LAST["mean_exec_time_ns"] = res.mean_exec_time_ns
    LAST["per_core_scope_times"] = res.per_core_scope_times
    LAST["trace_path"] = (res.instructions_and_trace or (None, None))[1]
    LAST["profile_json"] = res.profile_json

    zs = np.stack([res.results[c]["z_out"] for c in range(pre["n_cores"])])
    z = zs[pre["core_of"], pre["slot_in_core"]]
    return z.astype(np.float32)


if __name__ == "__main__":
    # small smoke test against numpy reference
    rng = np.random.default_rng(0)
    N, E, D, H = 4096, 40000, 128, 512
    ei = rng.integers(0, N, size=(2, E)).astype(np.int64)
    x = rng.standard_normal((N, D), dtype=np.float32)
    W1 = rng.standard_normal((D, H), dtype=np.float32) / np.sqrt(D)
    b1 = rng.standard_normal(H).astype(np.float32) * 0.1
    W2 = rng.standard_normal((H, D), dtype=np.float32) / np.sqrt(H)
    b2 = rng.standard_normal(D).astype(np.float32) * 0.1

    deg = np.bincount(ei[1], minlength=N) + 1.0
    dinv = 1.0 / np.sqrt(deg)
    asrc = np.concatenate([ei[0], np.arange(N)])
    adst = np.concatenate([ei[1], np.arange(N)])
    nrm = dinv[asrc] * dinv[adst]

    def agg(t):
        out = np.zeros_like(t)
        np.add.at(out, adst, t[asrc] * nrm[:, None])
        return out

    z1 = np.maximum(agg(x.astype(np.float64)) @ W1 + b1, 0)
    ref = agg(z1 @ W2) + b2

    got = kernel(x, W1, b1, W2, b2, ei)
    err = np.abs(got - ref)
    rel = err.max() / np.abs(ref).max()
    print(f"exec_time_ns: {LAST['exec_time_ns']}")
    print(f"max abs err {err.max():.3e}  rel(absmax) {rel:.3e}")
